# revision 10
# baseline (speedup 1.0000x reference)
"""Differential attention kernel for 8 Trainium2 NeuronCores.

Sharding: core c handles batch b = c//4, query rows [(c%4)*1024, (c%4+1)*1024).
Each core receives x[b]^T (bf16, key-columns rolled so its own query block is
first), computes K^T/Q^T projections W-stationary, V x-stationary, applies RoPE
on the transposed layout, runs both softmax branches with fused denominators
(activation accum_out), combines P = E1 - (lam*d1/d2)*E2, transposes P via the
DMA XBAR, and accumulates P@V on the tensor engine.

Schedule: the scalar-engine exp stream is the bottleneck (~78us/core), so
projections are interleaved with attention per key-quarter to start the exp
stream as early as possible and keep it gapless: Q/K-quarter-0 projections run
in column halves as their DMA slices land, then per quarter h the current
q-group's QK+exp runs while quarter h+1 projects.  Attention runs in 2 groups
of 4 q-tiles (SBUF limit on the exp tiles).  Combine work is split DVE/Pool
to keep both under the scalar-engine time.
"""

import math
from contextlib import ExitStack

import ml_dtypes
import numpy as np

import concourse.bass as bass
import concourse.mybir as mybir
import concourse.tile as tile
from concourse import bacc
from concourse.bass_utils import run_bass_kernel_spmd

B, S, D = 2, 4096, 1024
HD = 64
ROT = 128
NQ = 1024  # query rows per core
N_CORES = 8
DC = D // 128  # contraction chunks for the projections
KQ = 1024  # keys per quarter
NKQ = S // KQ  # 4 quarters
NMAX = 512  # moving-operand cap
GRP = 4  # q-tiles per attention group
FP32 = mybir.dt.float32
BF16 = mybir.dt.bfloat16
AF = mybir.ActivationFunctionType
ALU = mybir.AluOpType

_prog_cache = {}


def _build_kernel(ctx: ExitStack, tc, xT, w_sb_aps, cosT, sinT, lamn, out):
    nc = tc.nc

    const = ctx.enter_context(tc.tile_pool(name="const", bufs=1))
    xt_pool = ctx.enter_context(tc.tile_pool(name="xt", bufs=3))

    xTr = xT.rearrange("(a p) s -> p a s", p=128)
    w_sb = {}
    cos_sb = const.tile([128, S], BF16, tag="cos")
    sin_sb = const.tile([128, S], BF16, tag="sin")
    lam_sb = const.tile([128, 1], FP32, tag="lam")
    kT = [const.tile([128, KQ], BF16, tag=f"kT{h}", name=f"kT{h}") for h in range(NKQ)]
    qT = const.tile([128, NQ], BF16, tag="qT")
    v_sb = const.tile([128, S * ROT // 128], BF16, tag="v")  # flat [128, 4096]

    xt_q = [None] * NKQ
    xt_q[0] = xt_pool.tile([128, DC, KQ], BF16, tag="xt", name="xt0")

    # --- input DMAs, ordered so quarter-0 work starts ASAP ---
    def load_w(name):
        t = const.tile([128, DC, ROT], BF16, tag=name, name=name)
        nc.sync.dma_start(t[:], w_sb_aps[name].rearrange("(a p) r -> p a r", p=128))
        w_sb[name] = t

    load_w("wk")
    nc.sync.dma_start(xt_q[0][:, :, 0:512], xTr[:, :, 0:512])
    load_w("wq")
    nc.sync.dma_start(cos_sb[:, 0:512], cosT[:, 0:512])
    nc.sync.dma_start(sin_sb[:, 0:512], sinT[:, 0:512])
    nc.sync.dma_start(xt_q[0][:, :, 512:1024], xTr[:, :, 512:1024])
    nc.sync.dma_start(cos_sb[:, 512:1024], cosT[:, 512:1024])
    nc.sync.dma_start(sin_sb[:, 512:1024], sinT[:, 512:1024])
    load_w("wv")
    nc.sync.dma_start(lam_sb[:], lamn)
    for h in range(1, NKQ):
        xt_q[h] = xt_pool.tile([128, DC, KQ], BF16, tag="xt", name=f"xt{h}")
        ksl = slice(h * KQ, (h + 1) * KQ)
        nc.sync.dma_start(xt_q[h][:], xTr[:, :, ksl])
        nc.sync.dma_start(cos_sb[:, ksl], cosT[:, ksl])
        nc.sync.dma_start(sin_sb[:, ksl], sinT[:, ksl])

    # pools (PSUM: 3 score slots x2 banks + 2 PV banks = 8 banks)
    ps = ctx.enter_context(tc.tile_pool(name="ps", bufs=1, space="PSUM"))
    pso = ctx.enter_context(tc.tile_pool(name="pso", bufs=1, space="PSUM"))
    ropetmp = ctx.enter_context(tc.tile_pool(name="ropetmp", bufs=2))
    epool = ctx.enter_context(tc.tile_pool(name="epool", bufs=1))
    dstat = ctx.enter_context(tc.tile_pool(name="dstat", bufs=4))
    work = ctx.enter_context(tc.tile_pool(name="work", bufs=2))

    sidx = [0]

    def salloc(name):
        t = ps.tile([128, KQ], FP32, tag=f"s{sidx[0] % 3}", name=name)
        sidx[0] += 1
        return t

    # prime the Exp table on the scalar engine before the critical stream
    warm = dstat.tile([128, 1], FP32, tag="warm")
    nc.scalar.activation(warm[:], lam_sb[:], AF.Exp)

    def rope(dst, src_ps, s_off, width):
        csl = slice(s_off, s_off + width)
        t1 = ropetmp.tile([128, KQ], FP32, tag="t1")
        t2 = ropetmp.tile([128, KQ], FP32, tag="t2")
        nc.vector.tensor_mul(t1[:, 0:width], src_ps[:], cos_sb[:, csl])
        nc.vector.tensor_mul(t2[0:64, 0:width], src_ps[64:128, :], sin_sb[0:64, csl])
        nc.vector.tensor_mul(t2[64:128, 0:width], src_ps[0:64, :], sin_sb[64:128, csl])
        nc.gpsimd.tensor_add(dst, t1[:, 0:width], t2[:, 0:width])

    def proj_cols(dst_ps, w_tile, xq, c0, width):
        """dst_ps[:, 0:width] = W^T x for key-columns [c0, c0+width)."""
        for nn in range(width // NMAX):
            for dc in range(DC):
                nc.tensor.matmul(
                    dst_ps[:, nn * NMAX : (nn + 1) * NMAX],
                    lhsT=w_tile[:, dc, :],
                    rhs=xq[:, dc, c0 + nn * NMAX : c0 + (nn + 1) * NMAX],
                    start=(dc == 0),
                    stop=(dc == DC - 1),
                )

    # K-quarter-0 first (critical path to the first exp), in column halves
    # that unblock as their xt slices land; then the group-0 query half
    pk0 = salloc("pk0")
    for half in range(2):
        c0 = half * 512
        proj_cols(pk0[:, c0 : c0 + 512], w_sb["wk"], xt_q[0], c0, 512)
        rope(kT[0][:, c0 : c0 + 512], pk0[:, c0 : c0 + 512], c0, 512)

    def qproj(half):
        c0 = half * 512
        pq = salloc(f"pq{half}")
        proj_cols(pq[:, 0:512], w_sb["wq"], xt_q[0], c0, 512)
        rope(qT[:, c0 : c0 + 512], pq[:, 0:512], c0, 512)

    qproj(0)

    def kproj(h):
        pk = salloc(f"pk{h}")
        proj_cols(pk[:, :], w_sb["wk"], xt_q[h], 0, KQ)
        rope(kT[h][:, :], pk[:, :], h * KQ, KQ)

    def vproj(h):
        # V for quarter h: 8 key-chunks of 128 rows accumulated into one
        # rotation slot, drained by two [128,512] copies
        pv = salloc(f"pv{h}")
        for sub in range(2):
            for c in range(4):
                so = sub * 512 + c * 128
                for dc in range(DC):
                    nc.tensor.matmul(
                        pv[:, so : so + 128],
                        lhsT=xt_q[h][:, dc, so : so + 128],
                        rhs=w_sb["wv"][:, dc, :],
                        start=(dc == 0),
                        stop=(dc == DC - 1),
                    )
            o0 = (h * 8 + sub * 4) * 128
            nc.vector.tensor_copy(
                v_sb[:, o0 : o0 + 512], pv[:, sub * 512 : (sub + 1) * 512]
            )

    e1 = {}
    e2 = {}
    d1p = {}
    d2p = {}

    def qk_exp_qt(qt, h):
        if True:
            if h == 0:
                e1[qt] = epool.tile([128, S], BF16, tag=f"e1_{qt % GRP}", name=f"e1_{qt}")
                e2[qt] = epool.tile([128, S], BF16, tag=f"e2_{qt % GRP}", name=f"e2_{qt}")
                d1p[qt] = dstat.tile([128, NKQ], FP32, tag=f"d1p_{qt % GRP}", name=f"d1p_{qt}")
                d2p[qt] = dstat.tile([128, NKQ], FP32, tag=f"d2p_{qt % GRP}", name=f"d2p_{qt}")
            qsl = slice(qt * 128, (qt + 1) * 128)
            for br in range(2):
                psc = salloc(f"psc{qt}_{h}_{br}")
                lo, hi = (0, 64) if br == 0 else (64, 128)
                for nn in range(KQ // NMAX):
                    nc.tensor.matmul(
                        psc[:, nn * NMAX : (nn + 1) * NMAX],
                        lhsT=qT[lo:hi, qsl],
                        rhs=kT[h][lo:hi, nn * NMAX : (nn + 1) * NMAX],
                        start=True,
                        stop=True,
                        tile_position=(br * 64, 0),
                    )
                e_t, d_t = (e1[qt], d1p[qt]) if br == 0 else (e2[qt], d2p[qt])
                nc.scalar.activation(
                    e_t[:, h * KQ : (h + 1) * KQ], psc[:], AF.Exp,
                    scale=HD**-0.5, accum_out=d_t[:, h : h + 1],
                )

    def qk_exp(g, h):
        for qt in range(g * GRP, (g + 1) * GRP):
            qk_exp_qt(qt, h)

    def finish_qt(qt):
        d1 = dstat.tile([128, 1], FP32, tag="d1")
        d2 = dstat.tile([128, 1], FP32, tag="d2")
        nc.vector.reduce_sum(d1[:], d1p[qt][:], axis=mybir.AxisListType.X)
        nc.vector.reduce_sum(d2[:], d2p[qt][:], axis=mybir.AxisListType.X)
        r1 = dstat.tile([128, 1], FP32, tag="r1")
        r2 = dstat.tile([128, 1], FP32, tag="r2")
        nc.vector.reciprocal(r1[:], d1[:])
        nc.vector.reciprocal(r2[:], d2[:])
        c2n = dstat.tile([128, 1], FP32, tag="c2n")
        nc.vector.tensor_mul(c2n[:], d1[:], r2[:])
        nc.vector.tensor_mul(c2n[:], c2n[:], lam_sb[:])

        # combine + transpose + PV per k-half (combine on DVE; Pool can't
        # run TensorScalarPtr)
        pso_t = pso.tile([128, ROT], FP32, tag=f"o{qt % 2}", name=f"pso{qt}")
        for th in range(2):
            sl = slice(th * (S // 2), (th + 1) * (S // 2))
            p_t = work.tile([128, S // 2], BF16, tag=f"p{th}", name=f"p{th}_{qt}")
            nc.vector.scalar_tensor_tensor(
                out=p_t[:], in0=e2[qt][:, sl], scalar=c2n[:],
                in1=e1[qt][:, sl], op0=ALU.mult, op1=ALU.add,
            )
            pT = work.tile([128, S // 256, 128], BF16, tag=f"pT{th}", name=f"pT{th}_{qt}")
            nc.sync.dma_start(pT[:], p_t[:], transpose=True)
            for kc in range(th * 16, th * 16 + 16):
                nc.tensor.matmul(
                    pso_t[:],
                    lhsT=pT[:, kc % 16, :],
                    rhs=v_sb[:, kc * ROT : (kc + 1) * ROT],
                    start=(kc == 0),
                    stop=(kc == S // 128 - 1),
                )
        o_t = work.tile([128, ROT], FP32, tag="o")
        nc.vector.tensor_scalar_mul(o_t[:], pso_t[:], r1[:])
        # SWDGE path keeps the small output stores off the HWDGE xbar
        nc.gpsimd.dma_start(out[qt * 128 : (qt + 1) * 128, :], o_t[:])

    # ---- schedule ----
    # group 0 h-major with K-projection one window ahead; group 1 qt-major so
    # each finish chain overlaps the remaining exp stream
    qk_exp(0, 0)
    qproj(1)
    kproj(1)
    vproj(0)
    qk_exp(0, 1)
    kproj(2)
    vproj(1)
    qk_exp(0, 2)
    kproj(3)
    vproj(2)
    qk_exp(0, 3)
    for h in range(NKQ):
        qk_exp_qt(4, h)
    vproj(3)
    finish_qt(0)
    finish_qt(1)
    for h in range(NKQ):
        qk_exp_qt(5, h)
    finish_qt(2)
    finish_qt(3)
    for h in range(NKQ):
        qk_exp_qt(6, h)
    finish_qt(4)
    for h in range(NKQ):
        qk_exp_qt(7, h)
    finish_qt(5)
    finish_qt(6)
    finish_qt(7)


def _get_program(repeat=1):
    if repeat in _prog_cache:
        return _prog_cache[repeat]
    nc = bacc.Bacc("TRN2", target_bir_lowering=False, debug=False, num_devices=N_CORES)
    xT = nc.dram_tensor("xT", [D, S], BF16, kind="ExternalInput").ap()
    wq = nc.dram_tensor("wq", [D, ROT], BF16, kind="ExternalInput").ap()
    wk = nc.dram_tensor("wk", [D, ROT], BF16, kind="ExternalInput").ap()
    wv = nc.dram_tensor("wv", [D, ROT], BF16, kind="ExternalInput").ap()
    cosT = nc.dram_tensor("cosT", [ROT, S], BF16, kind="ExternalInput").ap()
    sinT = nc.dram_tensor("sinT", [ROT, S], BF16, kind="ExternalInput").ap()
    lamn = nc.dram_tensor("lamn", [128, 1], FP32, kind="ExternalInput").ap()
    out = nc.dram_tensor("out", [NQ, ROT], FP32, kind="ExternalOutput").ap()

    with tile.TileContext(nc) as tc:
        for rep in range(repeat):
            if rep > 0:
                # isolate repeated bodies so timing slopes measure single-run latency
                tc.strict_bb_all_engine_barrier()
            with ExitStack() as ctx:
                _build_kernel(
                    ctx, tc, xT, {"wq": wq, "wk": wk, "wv": wv}, cosT, sinT, lamn, out
                )
    nc.compile()
    _prog_cache[repeat] = nc
    return nc


def make_in_maps(x, Wq, Wk, Wv, lambda_q1, lambda_q2, lambda_k1, lambda_k2):
    x = np.asarray(x, dtype=np.float32)
    Wq, Wk, Wv = (np.asarray(w, dtype=np.float32) for w in (Wq, Wk, Wv))
    lq1, lq2, lk1, lk2 = (
        np.asarray(v, dtype=np.float32)
        for v in (lambda_q1, lambda_q2, lambda_k1, lambda_k2)
    )

    lam_init = 0.8 - 0.6 * math.exp(-0.3 * 1)
    lam = float(
        np.exp(np.sum(lq1 * lk1)) - np.exp(np.sum(lq2 * lk2)) + lam_init
    )

    inv = 1.0 / (10000.0 ** (np.arange(0, ROT, 2, dtype=np.float32) / ROT))
    freqs = np.arange(S, dtype=np.float32)[:, None] * inv[None, :]  # [S, 64]
    cosh = np.cos(freqs)
    sinh = np.sin(freqs)
    cosT_full = np.concatenate([cosh, cosh], axis=1).T  # [128, S]
    sinT_full = np.concatenate([-sinh, sinh], axis=1).T

    bf = ml_dtypes.bfloat16
    wq_b, wk_b, wv_b = (np.ascontiguousarray(w, dtype=bf) for w in (Wq, Wk, Wv))
    lam_arr = np.full((128, 1), -lam, dtype=np.float32)

    in_maps = []
    for c in range(N_CORES):
        b, qoff = c // 4, (c % 4) * NQ
        xTr = np.roll(x[b].T, -qoff, axis=1)
        in_maps.append(
            {
                "xT": np.ascontiguousarray(xTr, dtype=bf),
                "wq": wq_b, "wk": wk_b, "wv": wv_b,
                "cosT": np.ascontiguousarray(np.roll(cosT_full, -qoff, axis=1), dtype=bf),
                "sinT": np.ascontiguousarray(np.roll(sinT_full, -qoff, axis=1), dtype=bf),
                "lamn": lam_arr,
            }
        )
    return in_maps


def assemble_out(results):
    outs = [np.asarray(results[c]["out"], dtype=np.float32) for c in range(N_CORES)]
    return np.stack(
        [np.concatenate(outs[0:4], axis=0), np.concatenate(outs[4:8], axis=0)]
    )


def kernel(x, Wq, Wk, Wv, lambda_q1, lambda_q2, lambda_k1, lambda_k2):
    in_maps = make_in_maps(x, Wq, Wk, Wv, lambda_q1, lambda_q2, lambda_k1, lambda_k2)
    nc = _get_program()
    res = run_bass_kernel_spmd(nc, in_maps, list(range(N_CORES)))
    return assemble_out(res.results)
